# revision 31
# baseline (speedup 1.0000x reference)
"""AdaptiveSelection (topk_masking) Trainium2 kernel.

Per cluster c (8 clusters, one per NeuronCore, data parallel):
  Q  = feats @ q_w.T + q_b             [N, 128]
  qk = key @ q_w.T + q_b               [1, 128]
  s  = Q @ qk.T / sqrt(128)            [N]     (scores)
  A  = softmax(s)                      [N]
  idx = top_k(A, 128)                  (descending order)
  selected = feats[idx]                [128, D]
  fusion = A.T @ (feats @ v_w.T + v_b) [D]

Device restructurings:
  * s = feats @ w + const, w = q_w.T @ qk — the const and 1/sqrt(128) scale
    do not change the ordering, and softmax shift-invariance kills the const;
    the scale is applied inside the exp activation.
  * fusion = (sum_j e_j feats_j / sum_j e_j) @ v_w.T + v_b with e = exp(s/c):
    the e-weighted feature sum accumulates in PSUM (bf16 operands) during
    streaming, so the 4096x1024x1024 V matmul disappears. The final
    (a/z) @ v_w.T runs on the PE against a host-pretransposed bf16 v_w.T.
  * top-128: per-partition top-k (vector.max/max_index on a [128, 32] score
    layout, 32 elements per partition -> empirically max 5 of the global
    top-128 share a partition; 6 kept for margin), exact global ranks of the
    768 candidates via compare+accumulate, then a one-hot x index matmul
    yields the 128 row ids in descending-score order, and one indirect DMA
    gathers those rows from HBM.
  * feats stream as [128 partitions x 2 rows x 1024] groups so each DMA
    descriptor covers 8KB contiguous on both sides.
"""

import numpy as np

import concourse.mybir as mybir
from concourse import bacc, bass, tile
from concourse.bass_utils import run_bass_kernel_spmd

NCORES = 8
NPER = 4096
DIM = 1024
QDIM = 128
TOPK = 128
P = 128
R = 2  # feature rows per partition per streamed group
NG = NPER // (P * R)  # 16 streamed groups
NT = NPER // P  # 32 score columns; col = R*g + r, global row = 256g + 2p + r
NCPP = 6  # candidates kept per partition (empirical max in top-128 is 5)
W = P * NCPP  # 768 candidates
F32 = mybir.dt.float32
BF16 = mybir.dt.bfloat16
U32 = mybir.dt.uint32

_CACHE = {}


def build_bass():
    nc = bacc.Bacc(None, target_bir_lowering=False)

    feats_e = nc.declare_dram_parameter("feats", [NPER, DIM], F32, isOutput=False)
    keycol_e = nc.declare_dram_parameter("keycol", [P, 8], F32, isOutput=False)
    qw_e = nc.declare_dram_parameter("qw", [QDIM, DIM], F32, isOutput=False)
    qwt_e = nc.declare_dram_parameter("qwt", [P, 8 * QDIM], F32, isOutput=False)
    qb_e = nc.declare_dram_parameter("qb", [QDIM, 1], F32, isOutput=False)
    # v_w.T in bf16, host-prepared: [i, o] layout so fusion contracts on PE
    vwt_e = nc.declare_dram_parameter("vwt", [DIM, DIM], BF16, isOutput=False)
    vb_e = nc.declare_dram_parameter("vb", [1, DIM], F32, isOutput=False)
    irow_e = nc.declare_dram_parameter("irow", [P, P], F32, isOutput=False)
    ciota2_e = nc.declare_dram_parameter("ciota2", [P, 1], U32, isOutput=False)
    outsel_e = nc.declare_dram_parameter("out_sel", [TOPK, DIM], F32, isOutput=True)
    outfus_e = nc.declare_dram_parameter("out_fus", [1, DIM], F32, isOutput=True)

    mm = mybir.AluOpType.mult
    add = mybir.AluOpType.add
    is_gt = mybir.AluOpType.is_gt
    is_eq = mybir.AluOpType.is_equal
    COPY = mybir.ActivationFunctionType.Copy
    EXP = mybir.ActivationFunctionType.Exp

    with tile.TileContext(nc) as tc:
        with (
            tc.tile_pool(name="const", bufs=1) as cp,
            tc.tile_pool(name="stream", bufs=6) as sp,
            tc.tile_pool(name="scratch", bufs=3) as wp,
            tc.tile_pool(name="psum", bufs=2, space="PSUM") as pp,
            tc.tile_pool(name="psacc", bufs=1, space="PSUM") as pa,
        ):
            # ---------- setup: constants and small inputs ----------
            # critical-chain DMAs on the sync queue, the rest on gpsimd
            ones1 = cp.tile([1, P], F32)
            nc.vector.memset(ones1[:], 1.0)
            qwt_t = cp.tile([P, 8, QDIM], F32)
            nc.sync.dma_start(
                out=qwt_t[:].rearrange("p c q -> p (c q)"), in_=qwt_e[:, :]
            )
            keycol_t = cp.tile([P, 8], F32)
            nc.sync.dma_start(out=keycol_t[:], in_=keycol_e[:, :])
            qw_t = cp.tile([QDIM, DIM], F32)
            nc.sync.dma_start(out=qw_t[:], in_=qw_e[:, :])
            qb_t = cp.tile([QDIM, 1], F32)
            nc.sync.dma_start(out=qb_t[:], in_=qb_e[:, :])
            irow_t = cp.tile([P, P], F32)
            nc.gpsimd.dma_start(out=irow_t[:], in_=irow_e[:, :])
            ciota2_t = cp.tile([P, 1], U32)
            nc.gpsimd.dma_start(out=ciota2_t[:], in_=ciota2_e[:, :])
            vb_t = cp.tile([1, DIM], F32)
            nc.gpsimd.dma_start(out=vb_t[:], in_=vb_e[:, :])
            vwt_t = cp.tile([P, 8, DIM], BF16)
            for ib in range(8):
                nc.gpsimd.dma_start(
                    out=vwt_t[:, ib, :], in_=vwt_e[ib * P : (ib + 1) * P, :]
                )

            # qk = q_w @ key + q_b entirely on PE: contraction over dim via
            # the host-transposed q_w.T chunks and the column-chunked key
            pqk = pa.tile([P, 1], F32, tag="pz")
            for cch in range(8):
                nc.tensor.matmul(
                    out=pqk[:],
                    lhsT=qwt_t[:, cch, :],
                    rhs=keycol_t[:, cch : cch + 1],
                    start=(cch == 0),
                    stop=(cch == 7),
                )
            qk_t = cp.tile([QDIM, 1], F32)
            nc.vector.tensor_add(out=qk_t[:], in0=pqk[:], in1=qb_t[:])

            # w broadcast, duplicated for the 2-row groups: wb2[p, r, d] = w[d]
            wb2 = cp.tile([P, R, DIM], F32)
            for n in range(2):
                wps = pp.tile([P, 512], F32, tag="bc")
                nc.tensor.matmul(
                    out=wps[:],
                    lhsT=qk_t[:, 0:1].to_broadcast([QDIM, P]),
                    rhs=qw_t[:, n * 512 : (n + 1) * 512],
                    start=True,
                    stop=True,
                )
                for r in range(R):
                    nc.vector.tensor_copy(
                        out=wb2[:, r, n * 512 : (n + 1) * 512], in_=wps[:]
                    )

            # dummy indirect gather: absorbs the one-time GpSimd dynamic-DMA
            # setup (ucode load + queue drain, ~6us) during the streamed
            # phase instead of on the tail critical path
            zidx = cp.tile([16, 1], mybir.dt.int32)
            nc.vector.memset(zidx[:], 0)
            dummy = wp.tile([16, DIM], F32, tag="prod")
            nc.gpsimd.indirect_dma_start(
                out=dummy[:],
                out_offset=None,
                in_=feats_e[:, :],
                in_offset=bass.IndirectOffsetOnAxis(ap=zidx[:, 0:1], axis=0),
            )

            # ---------- streaming phase over 16 groups of 256 rows ----------
            # group g: partition p holds DRAM rows 256g + 2p + {0,1} (8KB
            # contiguous per partition on both sides of the DMA). Score
            # column for (g, r) is 2g + r.
            scores = cp.tile([P, NT], F32)
            evb = cp.tile([P, NT], BF16)
            pa0 = pa.tile([1, 512], F32, tag="pa0")
            pa1 = pa.tile([1, 512], F32, tag="pa1")
            pz = pa.tile([1, 1], F32, tag="pz")

            for g in range(NG):
                ft = sp.tile([P, R, DIM], F32, tag="feats")
                nc.sync.dma_start(
                    out=ft[:],
                    in_=feats_e[g * P * R : (g + 1) * P * R, :].rearrange(
                        "(p r) d -> p r d", r=R
                    ),
                )
                # bf16 cast split 3:1 between DVE and ACT to balance engines
                ftb = sp.tile([P, R, DIM], BF16, tag="featsb")
                nc.vector.tensor_copy(
                    out=ftb[:].rearrange("p r d -> p (r d)")[:, 0:1536],
                    in_=ft[:].rearrange("p r d -> p (r d)")[:, 0:1536],
                )
                nc.scalar.activation(
                    out=ftb[:].rearrange("p r d -> p (r d)")[:, 1536:2048],
                    in_=ft[:].rearrange("p r d -> p (r d)")[:, 1536:2048],
                    func=COPY,
                )
                prod = wp.tile([P, R, DIM], F32, tag="prod")
                nc.vector.tensor_tensor(out=prod[:], in0=ft[:], in1=wb2[:], op=mm)
                pact = wp.tile([P, R, DIM], F32, tag="actout")
                for r in range(R):
                    nc.scalar.activation(
                        out=pact[:, r, :],
                        in_=prod[:, r, :],
                        func=COPY,
                        accum_out=scores[:, R * g + r : R * g + r + 1],
                    )
                # softmax weight: exp(s / sqrt(QDIM)), batched over the group
                nc.scalar.activation(
                    out=evb[:, R * g : R * g + R],
                    in_=scores[:, R * g : R * g + R],
                    func=EXP,
                    scale=float(1.0 / np.sqrt(QDIM)),
                )
                # PSUM accumulation of e-weighted features (bf16 in, f32 acc)
                for r in range(R):
                    col = R * g + r
                    nc.tensor.matmul(
                        out=pa0[:],
                        lhsT=evb[:, col : col + 1],
                        rhs=ftb[:, r, 0:512],
                        start=(col == 0),
                        stop=(col == NT - 1),
                    )
                    nc.tensor.matmul(
                        out=pa1[:],
                        lhsT=evb[:, col : col + 1],
                        rhs=ftb[:, r, 512:1024],
                        start=(col == 0),
                        stop=(col == NT - 1),
                    )

            # ---------- fusion vector (placed early so PE/DMA overlap the
            # top-k phase): fusion = (a/z) @ v_w.T + v_b ----------
            zc = cp.tile([P, 1], F32)
            zact = wp.tile([P, NT], F32, tag="gts")
            nc.scalar.activation(out=zact[:], in_=evb[:], func=COPY, accum_out=zc[:])
            onescol = cp.tile([P, 1], F32)
            nc.vector.memset(onescol[:], 1.0)
            nc.tensor.matmul(
                out=pz[:], lhsT=zc[:], rhs=onescol[:], start=True, stop=True
            )
            rz = cp.tile([1, 1], F32)
            nc.vector.reciprocal(out=rz[:], in_=pz[:])
            a_sb = cp.tile([1, DIM], F32)
            nc.vector.tensor_scalar_mul(a_sb[:, 0:512], pa0[:], rz[:, 0:1])
            nc.vector.tensor_scalar_mul(a_sb[:, 512:1024], pa1[:], rz[:, 0:1])
            # a as a column-chunked [128, 8] layout (i = c*128 + p)
            acol = cp.tile([P, 8], F32)
            for c in range(8):
                nc.sync.dma_start(
                    out=acol[:, c : c + 1], in_=a_sb[:, c * P : (c + 1) * P]
                )
            acolb = cp.tile([P, 8], BF16)
            nc.vector.tensor_copy(out=acolb[:], in_=acol[:])
            pfus0 = pa.tile([1, 512], F32, tag="pfus0")
            pfus1 = pa.tile([1, 512], F32, tag="pfus1")
            for ib in range(8):
                nc.tensor.matmul(
                    out=pfus0[:],
                    lhsT=acolb[:, ib : ib + 1],
                    rhs=vwt_t[:, ib, 0:512],
                    start=(ib == 0),
                    stop=(ib == 7),
                )
                nc.tensor.matmul(
                    out=pfus1[:],
                    lhsT=acolb[:, ib : ib + 1],
                    rhs=vwt_t[:, ib, 512:1024],
                    start=(ib == 0),
                    stop=(ib == 7),
                )
            fus = cp.tile([1, DIM], F32)
            nc.vector.tensor_copy(out=fus[:, 0:512], in_=pfus0[:])
            nc.vector.tensor_copy(out=fus[:, 512:1024], in_=pfus1[:])
            nc.vector.tensor_add(out=fus[:], in0=fus[:], in1=vb_t[:])
            nc.sync.dma_start(out=outfus_e[:, :], in_=fus[:])

            # ---------- top-k: per-partition top-6 candidates ----------
            top8 = cp.tile([P, 8], F32)
            nc.vector.max(out=top8[:], in_=scores[:])
            idx8 = cp.tile([P, 8], U32)
            nc.vector.max_index(out=idx8[:], in_max=top8[:], in_values=scores[:])
            # global row id: col -> 256*(col>>1) + (col&1) + 2p
            gidx_u = cp.tile([P, 8], U32)
            sh = cp.tile([P, 8], U32)
            nc.vector.tensor_scalar(
                sh[:], idx8[:], 1, scalar2=None,
                op0=mybir.AluOpType.logical_shift_right,
            )
            nc.vector.tensor_scalar(
                sh[:], sh[:], 8, scalar2=None,
                op0=mybir.AluOpType.logical_shift_left,
            )
            nc.vector.tensor_scalar(
                gidx_u[:], idx8[:], 1, scalar2=None,
                op0=mybir.AluOpType.bitwise_and,
            )
            nc.vector.tensor_add(out=gidx_u[:], in0=gidx_u[:], in1=sh[:])
            nc.vector.tensor_add(
                out=gidx_u[:], in0=gidx_u[:], in1=ciota2_t[:, 0:1].to_broadcast([P, 8])
            )
            gidxf = cp.tile([P, 8], F32)
            nc.vector.tensor_copy(out=gidxf[:], in_=gidx_u[:])

            # ---------- exact global ranks of the W candidates ----------
            cf = cp.tile([1, W], F32)
            nc.sync.dma_start(out=cf[:], in_=top8[:, 0:NCPP])
            rs = cp.tile([P, W], F32)
            for n in range(2):
                lo = n * 512
                hi = min(W, lo + 512)
                if lo >= hi:
                    break
                rps = pp.tile([P, 512], F32, tag="bc")
                nc.tensor.matmul(
                    out=rps[:, 0 : hi - lo],
                    lhsT=ones1[:],
                    rhs=cf[:, lo:hi],
                    start=True,
                    stop=True,
                )
                nc.vector.tensor_copy(out=rs[:, lo:hi], in_=rps[:, 0 : hi - lo])
            # rank -> one-hot -> ordered-id matmul, pipelined per candidate col
            rank = cp.tile([P, NCPP], F32)
            po = pa.tile([P, 1], F32, tag="po")
            for c in range(NCPP):
                gts = wp.tile([P, W], F32, tag="gts")
                nc.vector.tensor_tensor(
                    out=gts[:],
                    in0=rs[:],
                    in1=top8[:, c : c + 1].to_broadcast([P, W]),
                    op=is_gt,
                )
                gact = wp.tile([P, W], F32, tag="gact")
                nc.scalar.activation(
                    out=gact[:], in_=gts[:], func=COPY,
                    accum_out=rank[:, c : c + 1],
                )
                oh = wp.tile([P, P], F32, tag="oh")
                nc.vector.tensor_tensor(
                    out=oh[:],
                    in0=rank[:, c : c + 1].to_broadcast([P, P]),
                    in1=irow_t[:],
                    op=is_eq,
                )
                nc.tensor.matmul(
                    out=po[:],
                    lhsT=oh[:],
                    rhs=gidxf[:, c : c + 1],
                    start=(c == 0),
                    stop=(c == NCPP - 1),
                )
            oidx = cp.tile([P, 1], mybir.dt.int32)
            nc.vector.tensor_copy(out=oidx[:], in_=po[:])

            # ---------- gather selected rows from HBM ----------
            sel = cp.tile([P, DIM], F32)
            nc.gpsimd.indirect_dma_start(
                out=sel[:],
                out_offset=None,
                in_=feats_e[:, :],
                in_offset=bass.IndirectOffsetOnAxis(ap=oidx[:, 0:1], axis=0),
            )
            nc.sync.dma_start(out=outsel_e[:, :], in_=sel[:])

    nc.finalize()
    return nc


def kernel(cluster_features, key_feats, q_w, q_b, v_w, v_b):
    import ml_dtypes

    cluster_features = np.ascontiguousarray(cluster_features, dtype=np.float32)
    key_feats = np.ascontiguousarray(key_feats, dtype=np.float32)
    q_w = np.ascontiguousarray(q_w, dtype=np.float32)
    q_b = np.ascontiguousarray(q_b, dtype=np.float32)
    v_w = np.ascontiguousarray(v_w, dtype=np.float32)
    v_b = np.ascontiguousarray(v_b, dtype=np.float32)

    if "nc" not in _CACHE:
        _CACHE["nc"] = build_bass()
    nc = _CACHE["nc"]

    qb_col = q_b.reshape(QDIM, 1).copy()
    qwt = np.ascontiguousarray(q_w.T.reshape(8, P, QDIM).transpose(1, 0, 2).reshape(P, 8 * QDIM))
    keycol = np.ascontiguousarray(key_feats[:, 0, :].reshape(NCORES, 8, P).transpose(0, 2, 1))
    vwt = np.ascontiguousarray(v_w.T).astype(ml_dtypes.bfloat16)
    vb_row = v_b.reshape(1, DIM).copy()
    irow = np.tile(np.arange(P, dtype=np.float32), (P, 1)).copy()
    ciota2 = (2 * np.arange(P, dtype=np.uint32)).reshape(P, 1).copy()

    in_maps = []
    for i in range(NCORES):
        in_maps.append(
            {
                "feats": cluster_features[i],
                "keycol": keycol[i],
                "qw": q_w,
                "qwt": qwt,
                "qb": qb_col,
                "vwt": vwt,
                "vb": vb_row,
                "irow": irow,
                "ciota2": ciota2,
            }
        )

    res = run_bass_kernel_spmd(nc, in_maps, core_ids=list(range(NCORES)))
    _CACHE["last_results"] = res

    selected = np.concatenate(
        [res.results[i]["out_sel"] for i in range(NCORES)], axis=0
    )
    fus = np.stack(
        [res.results[i]["out_fus"][0] for i in range(NCORES)], axis=0
    )
    return selected, fus


# revision 32
# speedup vs baseline: 1.1622x; 1.1622x over previous
"""AdaptiveSelection (topk_masking) Trainium2 kernel.

Per cluster c (8 clusters, one per NeuronCore, data parallel):
  Q  = feats @ q_w.T + q_b             [N, 128]
  qk = key @ q_w.T + q_b               [1, 128]
  s  = Q @ qk.T / sqrt(128)            [N]     (scores)
  A  = softmax(s)                      [N]
  idx = top_k(A, 128)                  (descending order)
  selected = feats[idx]                [128, D]
  fusion = A.T @ (feats @ v_w.T + v_b) [D]

Device restructurings:
  * s = feats @ w + const, w = q_w.T @ qk — the const and 1/sqrt(128) scale
    do not change the ordering, and softmax shift-invariance kills the const;
    the scale is applied inside the exp activation.
  * fusion = (sum_j e_j feats_j / sum_j e_j) @ v_w.T + v_b with e = exp(s/c):
    the e-weighted feature sum accumulates in PSUM (bf16 operands) during
    streaming, so the 4096x1024x1024 V matmul disappears. The final
    (a/z) @ v_w.T runs on the PE against a host-pretransposed bf16 v_w.T.
  * top-128: per-partition top-k (vector.max/max_index on a [128, 32] score
    layout, 32 elements per partition -> empirically max 5 of the global
    top-128 share a partition; 6 kept for margin), exact global ranks of the
    768 candidates via compare+accumulate, then a one-hot x index matmul
    yields the 128 row ids in descending-score order, and one indirect DMA
    gathers those rows from HBM.
  * feats stream as [128 partitions x 2 rows x 1024] groups so each DMA
    descriptor covers 8KB contiguous on both sides.
"""

import numpy as np

import concourse.mybir as mybir
from concourse import bacc, bass, tile
from concourse.bass_utils import run_bass_kernel_spmd

NCORES = 8
NPER = 4096
DIM = 1024
QDIM = 128
TOPK = 128
P = 128
R = 2  # feature rows per partition per streamed group
NG = NPER // (P * R)  # 16 streamed groups
NT = NPER // P  # 32 score columns; col = R*g + r, global row = 256g + 2p + r
NCPP = 6  # candidates kept per partition (empirical max in top-128 is 5)
W = P * NCPP  # 768 candidates
F32 = mybir.dt.float32
BF16 = mybir.dt.bfloat16
U32 = mybir.dt.uint32

_CACHE = {}


def build_bass():
    nc = bacc.Bacc(None, target_bir_lowering=False)

    feats_e = nc.declare_dram_parameter("feats", [NPER, DIM], F32, isOutput=False)
    keycol_e = nc.declare_dram_parameter("keycol", [P, 8], F32, isOutput=False)
    qw_e = nc.declare_dram_parameter("qw", [QDIM, DIM], F32, isOutput=False)
    qwt_e = nc.declare_dram_parameter("qwt", [P, 8 * QDIM], F32, isOutput=False)
    qb_e = nc.declare_dram_parameter("qb", [QDIM, 1], F32, isOutput=False)
    # v_w.T in bf16, host-prepared: [i, o] layout so fusion contracts on PE
    vwt_e = nc.declare_dram_parameter("vwt", [DIM, DIM], BF16, isOutput=False)
    vb_e = nc.declare_dram_parameter("vb", [1, DIM], F32, isOutput=False)
    irow_e = nc.declare_dram_parameter("irow", [P, P], F32, isOutput=False)
    ciota2_e = nc.declare_dram_parameter("ciota2", [P, 1], U32, isOutput=False)
    outsel_e = nc.declare_dram_parameter("out_sel", [TOPK, DIM], F32, isOutput=True)
    outfus_e = nc.declare_dram_parameter("out_fus", [1, DIM], F32, isOutput=True)

    mm = mybir.AluOpType.mult
    add = mybir.AluOpType.add
    is_gt = mybir.AluOpType.is_gt
    is_eq = mybir.AluOpType.is_equal
    COPY = mybir.ActivationFunctionType.Copy
    EXP = mybir.ActivationFunctionType.Exp

    with tile.TileContext(nc) as tc:
        with (
            tc.tile_pool(name="const", bufs=1) as cp,
            tc.tile_pool(name="stream", bufs=4) as sp,
            tc.tile_pool(name="prodp", bufs=4) as prp,
            tc.tile_pool(name="scratch", bufs=3) as wp,
            tc.tile_pool(name="psum", bufs=2, space="PSUM") as pp,
            tc.tile_pool(name="psacc", bufs=1, space="PSUM") as pa,
        ):
            # ---------- setup: constants and small inputs ----------
            # critical-chain DMAs on the sync queue, the rest on gpsimd
            ones1 = cp.tile([1, P], F32)
            nc.vector.memset(ones1[:], 1.0)
            qwt_t = cp.tile([P, 8, QDIM], F32)
            nc.sync.dma_start(
                out=qwt_t[:].rearrange("p c q -> p (c q)"), in_=qwt_e[:, :]
            )
            keycol_t = cp.tile([P, 8], F32)
            nc.sync.dma_start(out=keycol_t[:], in_=keycol_e[:, :])
            qw_t = cp.tile([QDIM, DIM], F32)
            nc.sync.dma_start(out=qw_t[:], in_=qw_e[:, :])
            qb_t = cp.tile([QDIM, 1], F32)
            nc.sync.dma_start(out=qb_t[:], in_=qb_e[:, :])
            irow_t = cp.tile([P, P], F32)
            nc.gpsimd.dma_start(out=irow_t[:], in_=irow_e[:, :])
            ciota2_t = cp.tile([P, 1], U32)
            nc.gpsimd.dma_start(out=ciota2_t[:], in_=ciota2_e[:, :])
            vb_t = cp.tile([1, DIM], F32)
            nc.gpsimd.dma_start(out=vb_t[:], in_=vb_e[:, :])
            vwt_t = cp.tile([P, 8, DIM], BF16)
            for ib in range(8):
                nc.gpsimd.dma_start(
                    out=vwt_t[:, ib, :], in_=vwt_e[ib * P : (ib + 1) * P, :]
                )

            # qk = q_w @ key + q_b entirely on PE: contraction over dim via
            # the host-transposed q_w.T chunks and the column-chunked key
            pqk = pa.tile([P, 1], F32, tag="pz")
            for cch in range(8):
                nc.tensor.matmul(
                    out=pqk[:],
                    lhsT=qwt_t[:, cch, :],
                    rhs=keycol_t[:, cch : cch + 1],
                    start=(cch == 0),
                    stop=(cch == 7),
                )
            qk_t = cp.tile([QDIM, 1], F32)
            nc.vector.tensor_add(out=qk_t[:], in0=pqk[:], in1=qb_t[:])

            # w broadcast, duplicated for the 2-row groups: wb2[p, r, d] = w[d]
            wb2 = cp.tile([P, R, DIM], F32)
            for n in range(2):
                wps = pp.tile([P, 512], F32, tag="bc")
                nc.tensor.matmul(
                    out=wps[:],
                    lhsT=qk_t[:, 0:1].to_broadcast([QDIM, P]),
                    rhs=qw_t[:, n * 512 : (n + 1) * 512],
                    start=True,
                    stop=True,
                )
                for r in range(R):
                    nc.vector.tensor_copy(
                        out=wb2[:, r, n * 512 : (n + 1) * 512], in_=wps[:]
                    )

            # dummy indirect gather: absorbs the one-time GpSimd dynamic-DMA
            # setup (ucode load + queue drain, ~6us) during the streamed
            # phase instead of on the tail critical path
            zidx = cp.tile([16, 1], mybir.dt.int32)
            nc.vector.memset(zidx[:], 0)
            dummy = cp.tile([16, DIM], F32)
            nc.gpsimd.indirect_dma_start(
                out=dummy[:],
                out_offset=None,
                in_=feats_e[:, :],
                in_offset=bass.IndirectOffsetOnAxis(ap=zidx[:, 0:1], axis=0),
            )

            # ---------- streaming phase over 16 groups of 256 rows ----------
            # group g: partition p holds DRAM rows 256g + 2p + {0,1} (8KB
            # contiguous per partition on both sides of the DMA). Score
            # column for (g, r) is 2g + r.
            scores = cp.tile([P, NT], F32)
            evb = cp.tile([P, NT], BF16)
            pa0 = pa.tile([1, 512], F32, tag="pa0")
            pa1 = pa.tile([1, 512], F32, tag="pa1")
            pz = pa.tile([1, 1], F32, tag="pz")

            for g in range(NG):
                ft = sp.tile([P, R, DIM], F32, tag="feats")
                nc.sync.dma_start(
                    out=ft[:],
                    in_=feats_e[g * P * R : (g + 1) * P * R, :].rearrange(
                        "(p r) d -> p r d", r=R
                    ),
                )
                ftb = sp.tile([P, R, DIM], BF16, tag="featsb")
                nc.vector.tensor_copy(out=ftb[:], in_=ft[:])
                prod = prp.tile([P, R, DIM], F32, tag="prod")
                nc.vector.tensor_tensor(out=prod[:], in0=ft[:], in1=wb2[:], op=mm)
                pact = wp.tile([P, R, DIM], F32, tag="actout")
                for r in range(R):
                    nc.scalar.activation(
                        out=pact[:, r, :],
                        in_=prod[:, r, :],
                        func=COPY,
                        accum_out=scores[:, R * g + r : R * g + r + 1],
                    )
                # softmax weight: exp(s / sqrt(QDIM)), batched over the group
                nc.scalar.activation(
                    out=evb[:, R * g : R * g + R],
                    in_=scores[:, R * g : R * g + R],
                    func=EXP,
                    scale=float(1.0 / np.sqrt(QDIM)),
                )
                # PSUM accumulation of e-weighted features (bf16 in, f32 acc)
                for r in range(R):
                    col = R * g + r
                    nc.tensor.matmul(
                        out=pa0[:],
                        lhsT=evb[:, col : col + 1],
                        rhs=ftb[:, r, 0:512],
                        start=(col == 0),
                        stop=(col == NT - 1),
                    )
                    nc.tensor.matmul(
                        out=pa1[:],
                        lhsT=evb[:, col : col + 1],
                        rhs=ftb[:, r, 512:1024],
                        start=(col == 0),
                        stop=(col == NT - 1),
                    )

            # ---------- fusion vector (placed early so PE/DMA overlap the
            # top-k phase): fusion = (a/z) @ v_w.T + v_b ----------
            zc = cp.tile([P, 1], F32)
            zact = wp.tile([P, NT], F32, tag="gts")
            nc.scalar.activation(out=zact[:], in_=evb[:], func=COPY, accum_out=zc[:])
            onescol = cp.tile([P, 1], F32)
            nc.vector.memset(onescol[:], 1.0)
            nc.tensor.matmul(
                out=pz[:], lhsT=zc[:], rhs=onescol[:], start=True, stop=True
            )
            rz = cp.tile([1, 1], F32)
            nc.vector.reciprocal(out=rz[:], in_=pz[:])
            a_sb = cp.tile([1, DIM], F32)
            nc.vector.tensor_scalar_mul(a_sb[:, 0:512], pa0[:], rz[:, 0:1])
            nc.vector.tensor_scalar_mul(a_sb[:, 512:1024], pa1[:], rz[:, 0:1])
            # a as a column-chunked [128, 8] layout (i = c*128 + p)
            acol = cp.tile([P, 8], F32)
            for c in range(8):
                nc.sync.dma_start(
                    out=acol[:, c : c + 1], in_=a_sb[:, c * P : (c + 1) * P]
                )
            acolb = cp.tile([P, 8], BF16)
            nc.vector.tensor_copy(out=acolb[:], in_=acol[:])
            pfus0 = pa.tile([1, 512], F32, tag="pfus0")
            pfus1 = pa.tile([1, 512], F32, tag="pfus1")
            for ib in range(8):
                nc.tensor.matmul(
                    out=pfus0[:],
                    lhsT=acolb[:, ib : ib + 1],
                    rhs=vwt_t[:, ib, 0:512],
                    start=(ib == 0),
                    stop=(ib == 7),
                )
                nc.tensor.matmul(
                    out=pfus1[:],
                    lhsT=acolb[:, ib : ib + 1],
                    rhs=vwt_t[:, ib, 512:1024],
                    start=(ib == 0),
                    stop=(ib == 7),
                )
            fus = cp.tile([1, DIM], F32)
            nc.vector.tensor_copy(out=fus[:, 0:512], in_=pfus0[:])
            nc.vector.tensor_copy(out=fus[:, 512:1024], in_=pfus1[:])
            nc.vector.tensor_add(out=fus[:], in0=fus[:], in1=vb_t[:])
            nc.sync.dma_start(out=outfus_e[:, :], in_=fus[:])

            # ---------- top-k: per-partition top-6 candidates ----------
            top8 = cp.tile([P, 8], F32)
            nc.vector.max(out=top8[:], in_=scores[:])
            idx8 = cp.tile([P, 8], U32)
            nc.vector.max_index(out=idx8[:], in_max=top8[:], in_values=scores[:])
            # global row id: col -> 256*(col>>1) + (col&1) + 2p
            gidx_u = cp.tile([P, 8], U32)
            sh = cp.tile([P, 8], U32)
            nc.vector.tensor_scalar(
                sh[:], idx8[:], 1, scalar2=None,
                op0=mybir.AluOpType.logical_shift_right,
            )
            nc.vector.tensor_scalar(
                sh[:], sh[:], 8, scalar2=None,
                op0=mybir.AluOpType.logical_shift_left,
            )
            nc.vector.tensor_scalar(
                gidx_u[:], idx8[:], 1, scalar2=None,
                op0=mybir.AluOpType.bitwise_and,
            )
            nc.vector.tensor_add(out=gidx_u[:], in0=gidx_u[:], in1=sh[:])
            nc.vector.tensor_add(
                out=gidx_u[:], in0=gidx_u[:], in1=ciota2_t[:, 0:1].to_broadcast([P, 8])
            )
            gidxf = cp.tile([P, 8], F32)
            nc.vector.tensor_copy(out=gidxf[:], in_=gidx_u[:])

            # ---------- exact global ranks of the W candidates ----------
            cf = cp.tile([1, W], F32)
            nc.sync.dma_start(out=cf[:], in_=top8[:, 0:NCPP])
            rs = cp.tile([P, W], F32)
            for n in range(2):
                lo = n * 512
                hi = min(W, lo + 512)
                if lo >= hi:
                    break
                rps = pp.tile([P, 512], F32, tag="bc")
                nc.tensor.matmul(
                    out=rps[:, 0 : hi - lo],
                    lhsT=ones1[:],
                    rhs=cf[:, lo:hi],
                    start=True,
                    stop=True,
                )
                nc.vector.tensor_copy(out=rs[:, lo:hi], in_=rps[:, 0 : hi - lo])
            # rank -> one-hot -> ordered-id matmul, pipelined per candidate col
            rank = cp.tile([P, NCPP], F32)
            po = pa.tile([P, 1], F32, tag="po")
            for c in range(NCPP):
                gts = wp.tile([P, W], F32, tag="gts")
                nc.vector.tensor_tensor(
                    out=gts[:],
                    in0=rs[:],
                    in1=top8[:, c : c + 1].to_broadcast([P, W]),
                    op=is_gt,
                )
                gact = wp.tile([P, W], F32, tag="gact")
                nc.scalar.activation(
                    out=gact[:], in_=gts[:], func=COPY,
                    accum_out=rank[:, c : c + 1],
                )
                oh = wp.tile([P, P], F32, tag="oh")
                nc.vector.tensor_tensor(
                    out=oh[:],
                    in0=rank[:, c : c + 1].to_broadcast([P, P]),
                    in1=irow_t[:],
                    op=is_eq,
                )
                nc.tensor.matmul(
                    out=po[:],
                    lhsT=oh[:],
                    rhs=gidxf[:, c : c + 1],
                    start=(c == 0),
                    stop=(c == NCPP - 1),
                )
            oidx = cp.tile([P, 1], mybir.dt.int32)
            nc.vector.tensor_copy(out=oidx[:], in_=po[:])

            # ---------- gather selected rows from HBM ----------
            sel = cp.tile([P, DIM], F32)
            nc.gpsimd.indirect_dma_start(
                out=sel[:],
                out_offset=None,
                in_=feats_e[:, :],
                in_offset=bass.IndirectOffsetOnAxis(ap=oidx[:, 0:1], axis=0),
            )
            nc.sync.dma_start(out=outsel_e[:, :], in_=sel[:])

    nc.finalize()
    return nc


def kernel(cluster_features, key_feats, q_w, q_b, v_w, v_b):
    import ml_dtypes

    cluster_features = np.ascontiguousarray(cluster_features, dtype=np.float32)
    key_feats = np.ascontiguousarray(key_feats, dtype=np.float32)
    q_w = np.ascontiguousarray(q_w, dtype=np.float32)
    q_b = np.ascontiguousarray(q_b, dtype=np.float32)
    v_w = np.ascontiguousarray(v_w, dtype=np.float32)
    v_b = np.ascontiguousarray(v_b, dtype=np.float32)

    if "nc" not in _CACHE:
        _CACHE["nc"] = build_bass()
    nc = _CACHE["nc"]

    qb_col = q_b.reshape(QDIM, 1).copy()
    qwt = np.ascontiguousarray(q_w.T.reshape(8, P, QDIM).transpose(1, 0, 2).reshape(P, 8 * QDIM))
    keycol = np.ascontiguousarray(key_feats[:, 0, :].reshape(NCORES, 8, P).transpose(0, 2, 1))
    vwt = np.ascontiguousarray(v_w.T).astype(ml_dtypes.bfloat16)
    vb_row = v_b.reshape(1, DIM).copy()
    irow = np.tile(np.arange(P, dtype=np.float32), (P, 1)).copy()
    ciota2 = (2 * np.arange(P, dtype=np.uint32)).reshape(P, 1).copy()

    in_maps = []
    for i in range(NCORES):
        in_maps.append(
            {
                "feats": cluster_features[i],
                "keycol": keycol[i],
                "qw": q_w,
                "qwt": qwt,
                "qb": qb_col,
                "vwt": vwt,
                "vb": vb_row,
                "irow": irow,
                "ciota2": ciota2,
            }
        )

    res = run_bass_kernel_spmd(nc, in_maps, core_ids=list(range(NCORES)))
    _CACHE["last_results"] = res

    selected = np.concatenate(
        [res.results[i]["out_sel"] for i in range(NCORES)], axis=0
    )
    fus = np.stack(
        [res.results[i]["out_fus"][0] for i in range(NCORES)], axis=0
    )
    return selected, fus


# revision 33
# speedup vs baseline: 1.1697x; 1.0065x over previous
"""AdaptiveSelection (topk_masking) Trainium2 kernel.

Per cluster c (8 clusters, one per NeuronCore, data parallel):
  Q  = feats @ q_w.T + q_b             [N, 128]
  qk = key @ q_w.T + q_b               [1, 128]
  s  = Q @ qk.T / sqrt(128)            [N]     (scores)
  A  = softmax(s)                      [N]
  idx = top_k(A, 128)                  (descending order)
  selected = feats[idx]                [128, D]
  fusion = A.T @ (feats @ v_w.T + v_b) [D]

Device restructurings:
  * s = feats @ w + const, w = q_w.T @ qk — the const and 1/sqrt(128) scale
    do not change the ordering, and softmax shift-invariance kills the const;
    the scale is applied inside the exp activation.
  * fusion = (sum_j e_j feats_j / sum_j e_j) @ v_w.T + v_b with e = exp(s/c):
    the e-weighted feature sum accumulates in PSUM (bf16 operands) during
    streaming, so the 4096x1024x1024 V matmul disappears. The final
    (a/z) @ v_w.T runs on the PE against a host-pretransposed bf16 v_w.T.
  * top-128: per-partition top-k (vector.max/max_index on a [128, 32] score
    layout, 32 elements per partition -> empirically max 5 of the global
    top-128 share a partition; 6 kept for margin), exact global ranks of the
    768 candidates via compare+accumulate, then a one-hot x index matmul
    yields the 128 row ids in descending-score order, and one indirect DMA
    gathers those rows from HBM.
  * feats stream as [128 partitions x 2 rows x 1024] groups so each DMA
    descriptor covers 8KB contiguous on both sides.
"""

import numpy as np

import concourse.mybir as mybir
from concourse import bacc, bass, tile
from concourse.bass_utils import run_bass_kernel_spmd

NCORES = 8
NPER = 4096
DIM = 1024
QDIM = 128
TOPK = 128
P = 128
R = 2  # feature rows per partition per streamed group
NG = NPER // (P * R)  # 16 streamed groups
NT = NPER // P  # 32 score columns; col = R*g + r, global row = 256g + 2p + r
NCPP = 6  # candidates kept per partition (empirical max in top-128 is 5)
W = P * NCPP  # 768 candidates
F32 = mybir.dt.float32
BF16 = mybir.dt.bfloat16
U32 = mybir.dt.uint32

_CACHE = {}


def build_bass():
    nc = bacc.Bacc(None, target_bir_lowering=False)

    feats_e = nc.declare_dram_parameter("feats", [NPER, DIM], F32, isOutput=False)
    keycol_e = nc.declare_dram_parameter("keycol", [P, 8], F32, isOutput=False)
    qw_e = nc.declare_dram_parameter("qw", [QDIM, DIM], F32, isOutput=False)
    qwt_e = nc.declare_dram_parameter("qwt", [P, 8 * QDIM], F32, isOutput=False)
    qb_e = nc.declare_dram_parameter("qb", [QDIM, 1], F32, isOutput=False)
    # v_w.T in bf16, host-prepared: [i, o] layout so fusion contracts on PE
    vwt_e = nc.declare_dram_parameter("vwt", [DIM, DIM], BF16, isOutput=False)
    vb_e = nc.declare_dram_parameter("vb", [1, DIM], F32, isOutput=False)
    irow_e = nc.declare_dram_parameter("irow", [P, P], F32, isOutput=False)
    ciota2_e = nc.declare_dram_parameter("ciota2", [P, 1], U32, isOutput=False)
    outsel_e = nc.declare_dram_parameter("out_sel", [TOPK, DIM], F32, isOutput=True)
    outfus_e = nc.declare_dram_parameter("out_fus", [1, DIM], F32, isOutput=True)

    mm = mybir.AluOpType.mult
    add = mybir.AluOpType.add
    is_gt = mybir.AluOpType.is_gt
    is_eq = mybir.AluOpType.is_equal
    COPY = mybir.ActivationFunctionType.Copy
    EXP = mybir.ActivationFunctionType.Exp

    with tile.TileContext(nc) as tc:
        with (
            tc.tile_pool(name="const", bufs=1) as cp,
            tc.tile_pool(name="stream", bufs=4) as sp,
            tc.tile_pool(name="prodp", bufs=4) as prp,
            tc.tile_pool(name="scratch", bufs=3) as wp,
            tc.tile_pool(name="psum", bufs=2, space="PSUM") as pp,
            tc.tile_pool(name="psacc", bufs=1, space="PSUM") as pa,
        ):
            # ---------- setup: constants and small inputs ----------
            # critical-chain DMAs on the sync queue, the rest on gpsimd
            ones1 = cp.tile([1, P], F32)
            nc.vector.memset(ones1[:], 1.0)
            qwt_t = cp.tile([P, 8, QDIM], F32)
            nc.sync.dma_start(
                out=qwt_t[:].rearrange("p c q -> p (c q)"), in_=qwt_e[:, :]
            )
            keycol_t = cp.tile([P, 8], F32)
            nc.sync.dma_start(out=keycol_t[:], in_=keycol_e[:, :])
            qw_t = cp.tile([QDIM, DIM], F32)
            nc.sync.dma_start(out=qw_t[:], in_=qw_e[:, :])
            qb_t = cp.tile([QDIM, 1], F32)
            nc.sync.dma_start(out=qb_t[:], in_=qb_e[:, :])
            irow_t = cp.tile([P, P], F32)
            nc.gpsimd.dma_start(out=irow_t[:], in_=irow_e[:, :])
            ciota2_t = cp.tile([P, 1], U32)
            nc.gpsimd.dma_start(out=ciota2_t[:], in_=ciota2_e[:, :])
            vb_t = cp.tile([1, DIM], F32)
            nc.gpsimd.dma_start(out=vb_t[:], in_=vb_e[:, :])
            vwt_t = cp.tile([P, 8, DIM], BF16)
            for ib in range(8):
                nc.gpsimd.dma_start(
                    out=vwt_t[:, ib, :], in_=vwt_e[ib * P : (ib + 1) * P, :]
                )

            # qk = q_w @ key + q_b entirely on PE: contraction over dim via
            # the host-transposed q_w.T chunks and the column-chunked key
            pqk = pa.tile([P, 1], F32, tag="pz")
            for cch in range(8):
                nc.tensor.matmul(
                    out=pqk[:],
                    lhsT=qwt_t[:, cch, :],
                    rhs=keycol_t[:, cch : cch + 1],
                    start=(cch == 0),
                    stop=(cch == 7),
                )
            qk_t = cp.tile([QDIM, 1], F32)
            nc.vector.tensor_add(out=qk_t[:], in0=pqk[:], in1=qb_t[:])

            # w broadcast, duplicated for the 2-row groups: wb2[p, r, d] = w[d]
            wb2 = cp.tile([P, R, DIM], F32)
            for n in range(2):
                wps = pp.tile([P, 512], F32, tag="bc")
                nc.tensor.matmul(
                    out=wps[:],
                    lhsT=qk_t[:, 0:1].to_broadcast([QDIM, P]),
                    rhs=qw_t[:, n * 512 : (n + 1) * 512],
                    start=True,
                    stop=True,
                )
                for r in range(R):
                    nc.vector.tensor_copy(
                        out=wb2[:, r, n * 512 : (n + 1) * 512], in_=wps[:]
                    )

            # dummy indirect gather: absorbs the one-time GpSimd dynamic-DMA
            # setup (ucode load + queue drain, ~6us) during the streamed
            # phase instead of on the tail critical path
            zidx = cp.tile([16, 1], mybir.dt.int32)
            nc.vector.memset(zidx[:], 0)
            dummy = cp.tile([16, DIM], F32)
            nc.gpsimd.indirect_dma_start(
                out=dummy[:],
                out_offset=None,
                in_=feats_e[:, :],
                in_offset=bass.IndirectOffsetOnAxis(ap=zidx[:, 0:1], axis=0),
            )

            # ---------- streaming phase over 16 groups of 256 rows ----------
            # group g: partition p holds DRAM rows 256g + 2p + {0,1} (8KB
            # contiguous per partition on both sides of the DMA). Score
            # column for (g, r) is 2g + r.
            scores = cp.tile([P, NT], F32)
            evb = cp.tile([P, NT], BF16)
            pa0 = pa.tile([1, 512], F32, tag="pa0")
            pa1 = pa.tile([1, 512], F32, tag="pa1")
            pz = pa.tile([1, 1], F32, tag="pz")

            for g in range(NG):
                ft = sp.tile([P, R, DIM], F32, tag="feats")
                nc.sync.dma_start(
                    out=ft[:],
                    in_=feats_e[g * P * R : (g + 1) * P * R, :].rearrange(
                        "(p r) d -> p r d", r=R
                    ),
                )
                # bf16 cast split 3:1 between DVE and ACT to balance engines
                ftb = sp.tile([P, R, DIM], BF16, tag="featsb")
                nc.vector.tensor_copy(
                    out=ftb[:].rearrange("p r d -> p (r d)")[:, 0:1536],
                    in_=ft[:].rearrange("p r d -> p (r d)")[:, 0:1536],
                )
                nc.scalar.activation(
                    out=ftb[:].rearrange("p r d -> p (r d)")[:, 1536:2048],
                    in_=ft[:].rearrange("p r d -> p (r d)")[:, 1536:2048],
                    func=COPY,
                )
                prod = prp.tile([P, R, DIM], F32, tag="prod")
                nc.vector.tensor_tensor(out=prod[:], in0=ft[:], in1=wb2[:], op=mm)
                pact = wp.tile([P, R, DIM], F32, tag="actout")
                for r in range(R):
                    nc.scalar.activation(
                        out=pact[:, r, :],
                        in_=prod[:, r, :],
                        func=COPY,
                        accum_out=scores[:, R * g + r : R * g + r + 1],
                    )
                # softmax weight: exp(s / sqrt(QDIM)), batched over the group
                nc.scalar.activation(
                    out=evb[:, R * g : R * g + R],
                    in_=scores[:, R * g : R * g + R],
                    func=EXP,
                    scale=float(1.0 / np.sqrt(QDIM)),
                )
                # PSUM accumulation of e-weighted features (bf16 in, f32 acc)
                for r in range(R):
                    col = R * g + r
                    nc.tensor.matmul(
                        out=pa0[:],
                        lhsT=evb[:, col : col + 1],
                        rhs=ftb[:, r, 0:512],
                        start=(col == 0),
                        stop=(col == NT - 1),
                    )
                    nc.tensor.matmul(
                        out=pa1[:],
                        lhsT=evb[:, col : col + 1],
                        rhs=ftb[:, r, 512:1024],
                        start=(col == 0),
                        stop=(col == NT - 1),
                    )

            # ---------- fusion vector (placed early so PE/DMA overlap the
            # top-k phase): fusion = (a/z) @ v_w.T + v_b ----------
            zc = cp.tile([P, 1], F32)
            zact = wp.tile([P, NT], F32, tag="gts")
            nc.scalar.activation(out=zact[:], in_=evb[:], func=COPY, accum_out=zc[:])
            onescol = cp.tile([P, 1], F32)
            nc.vector.memset(onescol[:], 1.0)
            nc.tensor.matmul(
                out=pz[:], lhsT=zc[:], rhs=onescol[:], start=True, stop=True
            )
            rz = cp.tile([1, 1], F32)
            nc.vector.reciprocal(out=rz[:], in_=pz[:])
            a_sb = cp.tile([1, DIM], F32)
            nc.vector.tensor_scalar_mul(a_sb[:, 0:512], pa0[:], rz[:, 0:1])
            nc.vector.tensor_scalar_mul(a_sb[:, 512:1024], pa1[:], rz[:, 0:1])
            # a as a column-chunked [128, 8] layout (i = c*128 + p)
            acol = cp.tile([P, 8], F32)
            for c in range(8):
                nc.sync.dma_start(
                    out=acol[:, c : c + 1], in_=a_sb[:, c * P : (c + 1) * P]
                )
            acolb = cp.tile([P, 8], BF16)
            nc.vector.tensor_copy(out=acolb[:], in_=acol[:])
            pfus0 = pa.tile([1, 512], F32, tag="pfus0")
            pfus1 = pa.tile([1, 512], F32, tag="pfus1")
            for ib in range(8):
                nc.tensor.matmul(
                    out=pfus0[:],
                    lhsT=acolb[:, ib : ib + 1],
                    rhs=vwt_t[:, ib, 0:512],
                    start=(ib == 0),
                    stop=(ib == 7),
                )
                nc.tensor.matmul(
                    out=pfus1[:],
                    lhsT=acolb[:, ib : ib + 1],
                    rhs=vwt_t[:, ib, 512:1024],
                    start=(ib == 0),
                    stop=(ib == 7),
                )
            fus = cp.tile([1, DIM], F32)
            nc.vector.tensor_copy(out=fus[:, 0:512], in_=pfus0[:])
            nc.vector.tensor_copy(out=fus[:, 512:1024], in_=pfus1[:])
            nc.vector.tensor_add(out=fus[:], in0=fus[:], in1=vb_t[:])
            nc.sync.dma_start(out=outfus_e[:, :], in_=fus[:])

            # ---------- top-k: per-partition top-6 candidates ----------
            top8 = cp.tile([P, 8], F32)
            nc.vector.max(out=top8[:], in_=scores[:])
            idx8 = cp.tile([P, 8], U32)
            nc.vector.max_index(out=idx8[:], in_max=top8[:], in_values=scores[:])
            # global row id: col -> 256*(col>>1) + (col&1) + 2p
            gidx_u = cp.tile([P, 8], U32)
            sh = cp.tile([P, 8], U32)
            nc.vector.tensor_scalar(
                sh[:], idx8[:], 1, scalar2=None,
                op0=mybir.AluOpType.logical_shift_right,
            )
            nc.vector.tensor_scalar(
                sh[:], sh[:], 8, scalar2=None,
                op0=mybir.AluOpType.logical_shift_left,
            )
            nc.vector.tensor_scalar(
                gidx_u[:], idx8[:], 1, scalar2=None,
                op0=mybir.AluOpType.bitwise_and,
            )
            nc.vector.tensor_add(out=gidx_u[:], in0=gidx_u[:], in1=sh[:])
            nc.vector.tensor_add(
                out=gidx_u[:], in0=gidx_u[:], in1=ciota2_t[:, 0:1].to_broadcast([P, 8])
            )
            gidxf = cp.tile([P, 8], F32)
            nc.vector.tensor_copy(out=gidxf[:], in_=gidx_u[:])

            # ---------- exact global ranks of the W candidates ----------
            cf = cp.tile([1, W], F32)
            nc.sync.dma_start(out=cf[:], in_=top8[:, 0:NCPP])
            rs = cp.tile([P, W], F32)
            for n in range(2):
                lo = n * 512
                hi = min(W, lo + 512)
                if lo >= hi:
                    break
                rps = pp.tile([P, 512], F32, tag="bc")
                nc.tensor.matmul(
                    out=rps[:, 0 : hi - lo],
                    lhsT=ones1[:],
                    rhs=cf[:, lo:hi],
                    start=True,
                    stop=True,
                )
                nc.vector.tensor_copy(out=rs[:, lo:hi], in_=rps[:, 0 : hi - lo])
            # rank -> one-hot -> ordered-id matmul, pipelined per candidate col
            rank = cp.tile([P, NCPP], F32)
            po = pa.tile([P, 1], F32, tag="po")
            for c in range(NCPP):
                gts = wp.tile([P, W], F32, tag="gts")
                nc.vector.tensor_tensor(
                    out=gts[:],
                    in0=rs[:],
                    in1=top8[:, c : c + 1].to_broadcast([P, W]),
                    op=is_gt,
                )
                gact = wp.tile([P, W], F32, tag="gact")
                nc.scalar.activation(
                    out=gact[:], in_=gts[:], func=COPY,
                    accum_out=rank[:, c : c + 1],
                )
                oh = wp.tile([P, P], F32, tag="oh")
                nc.vector.tensor_tensor(
                    out=oh[:],
                    in0=rank[:, c : c + 1].to_broadcast([P, P]),
                    in1=irow_t[:],
                    op=is_eq,
                )
                nc.tensor.matmul(
                    out=po[:],
                    lhsT=oh[:],
                    rhs=gidxf[:, c : c + 1],
                    start=(c == 0),
                    stop=(c == NCPP - 1),
                )
            oidx = cp.tile([P, 1], mybir.dt.int32)
            nc.vector.tensor_copy(out=oidx[:], in_=po[:])

            # ---------- gather selected rows from HBM ----------
            sel = cp.tile([P, DIM], F32)
            nc.gpsimd.indirect_dma_start(
                out=sel[:],
                out_offset=None,
                in_=feats_e[:, :],
                in_offset=bass.IndirectOffsetOnAxis(ap=oidx[:, 0:1], axis=0),
            )
            nc.sync.dma_start(out=outsel_e[:, :], in_=sel[:])

    nc.finalize()
    return nc


def kernel(cluster_features, key_feats, q_w, q_b, v_w, v_b):
    import ml_dtypes

    cluster_features = np.ascontiguousarray(cluster_features, dtype=np.float32)
    key_feats = np.ascontiguousarray(key_feats, dtype=np.float32)
    q_w = np.ascontiguousarray(q_w, dtype=np.float32)
    q_b = np.ascontiguousarray(q_b, dtype=np.float32)
    v_w = np.ascontiguousarray(v_w, dtype=np.float32)
    v_b = np.ascontiguousarray(v_b, dtype=np.float32)

    if "nc" not in _CACHE:
        _CACHE["nc"] = build_bass()
    nc = _CACHE["nc"]

    qb_col = q_b.reshape(QDIM, 1).copy()
    qwt = np.ascontiguousarray(q_w.T.reshape(8, P, QDIM).transpose(1, 0, 2).reshape(P, 8 * QDIM))
    keycol = np.ascontiguousarray(key_feats[:, 0, :].reshape(NCORES, 8, P).transpose(0, 2, 1))
    vwt = np.ascontiguousarray(v_w.T).astype(ml_dtypes.bfloat16)
    vb_row = v_b.reshape(1, DIM).copy()
    irow = np.tile(np.arange(P, dtype=np.float32), (P, 1)).copy()
    ciota2 = (2 * np.arange(P, dtype=np.uint32)).reshape(P, 1).copy()

    in_maps = []
    for i in range(NCORES):
        in_maps.append(
            {
                "feats": cluster_features[i],
                "keycol": keycol[i],
                "qw": q_w,
                "qwt": qwt,
                "qb": qb_col,
                "vwt": vwt,
                "vb": vb_row,
                "irow": irow,
                "ciota2": ciota2,
            }
        )

    res = run_bass_kernel_spmd(nc, in_maps, core_ids=list(range(NCORES)))
    _CACHE["last_results"] = res

    selected = np.concatenate(
        [res.results[i]["out_sel"] for i in range(NCORES)], axis=0
    )
    fus = np.stack(
        [res.results[i]["out_fus"][0] for i in range(NCORES)], axis=0
    )
    return selected, fus


# revision 34
# speedup vs baseline: 1.1867x; 1.0145x over previous
"""AdaptiveSelection (topk_masking) Trainium2 kernel.

Per cluster c (8 clusters, one per NeuronCore, data parallel):
  Q  = feats @ q_w.T + q_b             [N, 128]
  qk = key @ q_w.T + q_b               [1, 128]
  s  = Q @ qk.T / sqrt(128)            [N]     (scores)
  A  = softmax(s)                      [N]
  idx = top_k(A, 128)                  (descending order)
  selected = feats[idx]                [128, D]
  fusion = A.T @ (feats @ v_w.T + v_b) [D]

Device restructurings:
  * s = feats @ w + const, w = q_w.T @ qk — the const and 1/sqrt(128) scale
    do not change the ordering, and softmax shift-invariance kills the const;
    the scale is applied inside the exp activation.
  * fusion = (sum_j e_j feats_j / sum_j e_j) @ v_w.T + v_b with e = exp(s/c):
    the e-weighted feature sum accumulates in PSUM (bf16 operands) during
    streaming, so the 4096x1024x1024 V matmul disappears. The final
    (a/z) @ v_w.T runs on the PE against a host-pretransposed bf16 v_w.T.
  * top-128: per-partition top-k (vector.max/max_index on a [128, 32] score
    layout, 32 elements per partition -> empirically max 5 of the global
    top-128 share a partition; 6 kept for margin), exact global ranks of the
    768 candidates via compare+accumulate, then a one-hot x index matmul
    yields the 128 row ids in descending-score order, and one indirect DMA
    gathers those rows from HBM.
  * feats stream as [128 partitions x 2 rows x 1024] groups so each DMA
    descriptor covers 8KB contiguous on both sides.
"""

import numpy as np

import concourse.mybir as mybir
from concourse import bacc, bass, tile
from concourse.bass_utils import run_bass_kernel_spmd

NCORES = 8
NPER = 4096
DIM = 1024
QDIM = 128
TOPK = 128
P = 128
R = 2  # feature rows per partition per streamed group
NG = NPER // (P * R)  # 16 streamed groups
NT = NPER // P  # 32 score columns; col = R*g + r, global row = 256g + 2p + r
NCPP = 6  # candidates kept per partition (empirical max in top-128 is 5)
W = P * NCPP  # 768 candidates
F32 = mybir.dt.float32
BF16 = mybir.dt.bfloat16
U32 = mybir.dt.uint32

_CACHE = {}


def build_bass():
    nc = bacc.Bacc(None, target_bir_lowering=False)

    feats_e = nc.declare_dram_parameter("feats", [NPER, DIM], F32, isOutput=False)
    keycol_e = nc.declare_dram_parameter("keycol", [P, 8], F32, isOutput=False)
    qw_e = nc.declare_dram_parameter("qw", [QDIM, DIM], F32, isOutput=False)
    qwt_e = nc.declare_dram_parameter("qwt", [P, 8 * QDIM], F32, isOutput=False)
    qb_e = nc.declare_dram_parameter("qb", [QDIM, 1], F32, isOutput=False)
    # v_w.T in bf16, host-prepared: [i, o] layout so fusion contracts on PE
    vwt_e = nc.declare_dram_parameter("vwt", [DIM, DIM], BF16, isOutput=False)
    vb_e = nc.declare_dram_parameter("vb", [1, DIM], F32, isOutput=False)
    irow_e = nc.declare_dram_parameter("irow", [P, P], F32, isOutput=False)
    ciota2_e = nc.declare_dram_parameter("ciota2", [P, 1], U32, isOutput=False)
    outsel_e = nc.declare_dram_parameter("out_sel", [TOPK, DIM], F32, isOutput=True)
    outfus_e = nc.declare_dram_parameter("out_fus", [1, DIM], F32, isOutput=True)

    mm = mybir.AluOpType.mult
    add = mybir.AluOpType.add
    is_gt = mybir.AluOpType.is_gt
    is_eq = mybir.AluOpType.is_equal
    COPY = mybir.ActivationFunctionType.Copy
    EXP = mybir.ActivationFunctionType.Exp

    with tile.TileContext(nc) as tc:
        with (
            tc.tile_pool(name="const", bufs=1) as cp,
            tc.tile_pool(name="stream", bufs=4) as sp,
            tc.tile_pool(name="prodp", bufs=4) as prp,
            tc.tile_pool(name="scratch", bufs=3) as wp,
            tc.tile_pool(name="psum", bufs=2, space="PSUM") as pp,
            tc.tile_pool(name="psacc", bufs=1, space="PSUM") as pa,
        ):
            # ---------- setup: constants and small inputs ----------
            # critical-chain DMAs on the sync queue, the rest on gpsimd
            ones1 = cp.tile([1, P], F32)
            nc.vector.memset(ones1[:], 1.0)
            qwt_t = cp.tile([P, 8, QDIM], F32)
            for hh in range(2):
                nc.sync.dma_start(
                    out=qwt_t[:].rearrange("p c q -> p (c q)")[:, hh * 512 : (hh + 1) * 512],
                    in_=qwt_e[:, hh * 512 : (hh + 1) * 512],
                )
            keycol_t = cp.tile([P, 8], F32)
            nc.sync.dma_start(out=keycol_t[:], in_=keycol_e[:, :])
            qw_t = cp.tile([QDIM, DIM], F32)
            for hh in range(2):
                nc.sync.dma_start(
                    out=qw_t[:, hh * 512 : (hh + 1) * 512],
                    in_=qw_e[:, hh * 512 : (hh + 1) * 512],
                )
            qb_t = cp.tile([QDIM, 1], F32)
            nc.sync.dma_start(out=qb_t[:], in_=qb_e[:, :])
            irow_t = cp.tile([P, P], F32)
            nc.gpsimd.dma_start(out=irow_t[:], in_=irow_e[:, :])
            ciota2_t = cp.tile([P, 1], U32)
            nc.gpsimd.dma_start(out=ciota2_t[:], in_=ciota2_e[:, :])
            vb_t = cp.tile([1, DIM], F32)
            nc.gpsimd.dma_start(out=vb_t[:], in_=vb_e[:, :])
            vwt_t = cp.tile([P, 8, DIM], BF16)
            for ib in range(8):
                nc.gpsimd.dma_start(
                    out=vwt_t[:, ib, :], in_=vwt_e[ib * P : (ib + 1) * P, :]
                )

            # qk = q_w @ key + q_b entirely on PE: contraction over dim via
            # the host-transposed q_w.T chunks and the column-chunked key
            pqk = pa.tile([P, 1], F32, tag="pz")
            for cch in range(8):
                nc.tensor.matmul(
                    out=pqk[:],
                    lhsT=qwt_t[:, cch, :],
                    rhs=keycol_t[:, cch : cch + 1],
                    start=(cch == 0),
                    stop=(cch == 7),
                )
            qk_t = cp.tile([QDIM, 1], F32)
            nc.vector.tensor_add(out=qk_t[:], in0=pqk[:], in1=qb_t[:])

            # w broadcast, duplicated for the 2-row groups: wb2[p, r, d] = w[d]
            wb2 = cp.tile([P, R, DIM], F32)
            for n in range(2):
                wps = pp.tile([P, 512], F32, tag="bc")
                nc.tensor.matmul(
                    out=wps[:],
                    lhsT=qk_t[:, 0:1].to_broadcast([QDIM, P]),
                    rhs=qw_t[:, n * 512 : (n + 1) * 512],
                    start=True,
                    stop=True,
                )
                for r in range(R):
                    nc.vector.tensor_copy(
                        out=wb2[:, r, n * 512 : (n + 1) * 512], in_=wps[:]
                    )

            # dummy indirect gather: absorbs the one-time GpSimd dynamic-DMA
            # setup (ucode load + queue drain, ~6us) during the streamed
            # phase instead of on the tail critical path
            zidx = cp.tile([16, 1], mybir.dt.int32)
            nc.vector.memset(zidx[:], 0)
            dummy = cp.tile([16, DIM], F32)
            nc.gpsimd.indirect_dma_start(
                out=dummy[:],
                out_offset=None,
                in_=feats_e[:, :],
                in_offset=bass.IndirectOffsetOnAxis(ap=zidx[:, 0:1], axis=0),
            )

            # ---------- streaming phase over 16 groups of 256 rows ----------
            # group g: partition p holds DRAM rows 256g + 2p + {0,1} (8KB
            # contiguous per partition on both sides of the DMA). Score
            # column for (g, r) is 2g + r.
            scores = cp.tile([P, NT], F32)
            evb = cp.tile([P, NT], BF16)
            pa0 = pa.tile([1, 512], F32, tag="pa0")
            pa1 = pa.tile([1, 512], F32, tag="pa1")
            pz = pa.tile([1, 1], F32, tag="pz")

            for g in range(NG):
                ft = sp.tile([P, R, DIM], F32, tag="feats")
                nc.sync.dma_start(
                    out=ft[:],
                    in_=feats_e[g * P * R : (g + 1) * P * R, :].rearrange(
                        "(p r) d -> p r d", r=R
                    ),
                )
                # bf16 cast split 3:1 between DVE and ACT to balance engines
                ftb = sp.tile([P, R, DIM], BF16, tag="featsb")
                nc.vector.tensor_copy(
                    out=ftb[:].rearrange("p r d -> p (r d)")[:, 0:1536],
                    in_=ft[:].rearrange("p r d -> p (r d)")[:, 0:1536],
                )
                nc.scalar.activation(
                    out=ftb[:].rearrange("p r d -> p (r d)")[:, 1536:2048],
                    in_=ft[:].rearrange("p r d -> p (r d)")[:, 1536:2048],
                    func=COPY,
                )
                prod = prp.tile([P, R, DIM], F32, tag="prod")
                nc.vector.tensor_tensor(out=prod[:], in0=ft[:], in1=wb2[:], op=mm)
                pact = wp.tile([P, R, DIM], F32, tag="actout")
                for r in range(R):
                    nc.scalar.activation(
                        out=pact[:, r, :],
                        in_=prod[:, r, :],
                        func=COPY,
                        accum_out=scores[:, R * g + r : R * g + r + 1],
                    )
                # softmax weight: exp(s / sqrt(QDIM)), batched over the group
                nc.scalar.activation(
                    out=evb[:, R * g : R * g + R],
                    in_=scores[:, R * g : R * g + R],
                    func=EXP,
                    scale=float(1.0 / np.sqrt(QDIM)),
                )
                # PSUM accumulation of e-weighted features (bf16 in, f32 acc)
                for r in range(R):
                    col = R * g + r
                    nc.tensor.matmul(
                        out=pa0[:],
                        lhsT=evb[:, col : col + 1],
                        rhs=ftb[:, r, 0:512],
                        start=(col == 0),
                        stop=(col == NT - 1),
                    )
                    nc.tensor.matmul(
                        out=pa1[:],
                        lhsT=evb[:, col : col + 1],
                        rhs=ftb[:, r, 512:1024],
                        start=(col == 0),
                        stop=(col == NT - 1),
                    )

            # ---------- fusion vector (placed early so PE/DMA overlap the
            # top-k phase): fusion = (a/z) @ v_w.T + v_b ----------
            zc = cp.tile([P, 1], F32)
            zact = wp.tile([P, NT], F32, tag="gts")
            nc.scalar.activation(out=zact[:], in_=evb[:], func=COPY, accum_out=zc[:])
            onescol = cp.tile([P, 1], F32)
            nc.vector.memset(onescol[:], 1.0)
            nc.tensor.matmul(
                out=pz[:], lhsT=zc[:], rhs=onescol[:], start=True, stop=True
            )
            rz = cp.tile([1, 1], F32)
            nc.vector.reciprocal(out=rz[:], in_=pz[:])
            a_sb = cp.tile([1, DIM], F32)
            nc.vector.tensor_scalar_mul(a_sb[:, 0:512], pa0[:], rz[:, 0:1])
            nc.vector.tensor_scalar_mul(a_sb[:, 512:1024], pa1[:], rz[:, 0:1])
            # a as a column-chunked [128, 8] layout (i = c*128 + p)
            acol = cp.tile([P, 8], F32)
            for c in range(8):
                nc.sync.dma_start(
                    out=acol[:, c : c + 1], in_=a_sb[:, c * P : (c + 1) * P]
                )
            acolb = cp.tile([P, 8], BF16)
            nc.vector.tensor_copy(out=acolb[:], in_=acol[:])
            pfus0 = pa.tile([1, 512], F32, tag="pfus0")
            pfus1 = pa.tile([1, 512], F32, tag="pfus1")
            for ib in range(8):
                nc.tensor.matmul(
                    out=pfus0[:],
                    lhsT=acolb[:, ib : ib + 1],
                    rhs=vwt_t[:, ib, 0:512],
                    start=(ib == 0),
                    stop=(ib == 7),
                )
                nc.tensor.matmul(
                    out=pfus1[:],
                    lhsT=acolb[:, ib : ib + 1],
                    rhs=vwt_t[:, ib, 512:1024],
                    start=(ib == 0),
                    stop=(ib == 7),
                )
            # ---------- top-k: per-partition top-6 candidates ----------
            top8 = cp.tile([P, 8], F32)
            nc.vector.max(out=top8[:], in_=scores[:])
            idx8 = cp.tile([P, 8], U32)
            nc.vector.max_index(out=idx8[:], in_max=top8[:], in_values=scores[:])
            # global row id: col -> 256*(col>>1) + (col&1) + 2p
            gidx_u = cp.tile([P, 8], U32)
            sh = cp.tile([P, 8], U32)
            nc.vector.tensor_scalar(
                sh[:], idx8[:], 1, scalar2=None,
                op0=mybir.AluOpType.logical_shift_right,
            )
            nc.vector.tensor_scalar(
                sh[:], sh[:], 8, scalar2=None,
                op0=mybir.AluOpType.logical_shift_left,
            )
            nc.vector.tensor_scalar(
                gidx_u[:], idx8[:], 1, scalar2=None,
                op0=mybir.AluOpType.bitwise_and,
            )
            nc.vector.tensor_add(out=gidx_u[:], in0=gidx_u[:], in1=sh[:])
            nc.vector.tensor_add(
                out=gidx_u[:], in0=gidx_u[:], in1=ciota2_t[:, 0:1].to_broadcast([P, 8])
            )
            gidxf = cp.tile([P, 8], F32)
            nc.vector.tensor_copy(out=gidxf[:], in_=gidx_u[:])

            # ---------- exact global ranks of the W candidates ----------
            cf = cp.tile([1, W], F32)
            nc.sync.dma_start(out=cf[:], in_=top8[:, 0:NCPP])
            rs = cp.tile([P, W], F32)
            for n in range(2):
                lo = n * 512
                hi = min(W, lo + 512)
                if lo >= hi:
                    break
                rps = pp.tile([P, 512], F32, tag="bc")
                nc.tensor.matmul(
                    out=rps[:, 0 : hi - lo],
                    lhsT=ones1[:],
                    rhs=cf[:, lo:hi],
                    start=True,
                    stop=True,
                )
                nc.vector.tensor_copy(out=rs[:, lo:hi], in_=rps[:, 0 : hi - lo])
            # rank -> one-hot -> ordered-id matmul, pipelined per candidate col
            rank = cp.tile([P, NCPP], F32)
            po = pa.tile([P, 1], F32, tag="po")
            for c in range(NCPP):
                gts = wp.tile([P, W], F32, tag="gts")
                nc.vector.tensor_tensor(
                    out=gts[:],
                    in0=rs[:],
                    in1=top8[:, c : c + 1].to_broadcast([P, W]),
                    op=is_gt,
                )
                gact = wp.tile([P, W], F32, tag="gact")
                nc.scalar.activation(
                    out=gact[:], in_=gts[:], func=COPY,
                    accum_out=rank[:, c : c + 1],
                )
                oh = wp.tile([P, P], F32, tag="oh")
                nc.vector.tensor_tensor(
                    out=oh[:],
                    in0=rank[:, c : c + 1].to_broadcast([P, P]),
                    in1=irow_t[:],
                    op=is_eq,
                )
                nc.tensor.matmul(
                    out=po[:],
                    lhsT=oh[:],
                    rhs=gidxf[:, c : c + 1],
                    start=(c == 0),
                    stop=(c == NCPP - 1),
                )
            oidx = cp.tile([P, 1], mybir.dt.int32)
            nc.vector.tensor_copy(out=oidx[:], in_=po[:])

            # ---------- gather selected rows from HBM (two half-gathers on
            # base-partition-0 tiles so transfers and writebacks overlap) ----
            oidx_b = cp.tile([64, 1], mybir.dt.int32)
            nc.sync.dma_start(out=oidx_b[:], in_=oidx[64:128, 0:1])
            sel_a = cp.tile([64, DIM], F32)
            sel_b = cp.tile([64, DIM], F32)
            nc.gpsimd.indirect_dma_start(
                out=sel_a[:],
                out_offset=None,
                in_=feats_e[:, :],
                in_offset=bass.IndirectOffsetOnAxis(ap=oidx[0:64, 0:1], axis=0),
            )
            nc.gpsimd.indirect_dma_start(
                out=sel_b[:],
                out_offset=None,
                in_=feats_e[:, :],
                in_offset=bass.IndirectOffsetOnAxis(ap=oidx_b[:, 0:1], axis=0),
            )
            for qq in range(2):
                nc.sync.dma_start(
                    out=outsel_e[qq * 32 : (qq + 1) * 32, :],
                    in_=sel_a[qq * 32 : (qq + 1) * 32, :],
                )
            for qq in range(2):
                nc.sync.dma_start(
                    out=outsel_e[64 + qq * 32 : 64 + (qq + 1) * 32, :],
                    in_=sel_b[qq * 32 : (qq + 1) * 32, :],
                )

            # fusion finalize (off the critical path, after the gather issue)
            fus = cp.tile([1, DIM], F32)
            nc.vector.tensor_copy(out=fus[:, 0:512], in_=pfus0[:])
            nc.vector.tensor_copy(out=fus[:, 512:1024], in_=pfus1[:])
            nc.vector.tensor_add(out=fus[:], in0=fus[:], in1=vb_t[:])
            nc.sync.dma_start(out=outfus_e[:, :], in_=fus[:])

    nc.finalize()
    return nc


def kernel(cluster_features, key_feats, q_w, q_b, v_w, v_b):
    import ml_dtypes

    cluster_features = np.ascontiguousarray(cluster_features, dtype=np.float32)
    key_feats = np.ascontiguousarray(key_feats, dtype=np.float32)
    q_w = np.ascontiguousarray(q_w, dtype=np.float32)
    q_b = np.ascontiguousarray(q_b, dtype=np.float32)
    v_w = np.ascontiguousarray(v_w, dtype=np.float32)
    v_b = np.ascontiguousarray(v_b, dtype=np.float32)

    if "nc" not in _CACHE:
        _CACHE["nc"] = build_bass()
    nc = _CACHE["nc"]

    qb_col = q_b.reshape(QDIM, 1).copy()
    qwt = np.ascontiguousarray(q_w.T.reshape(8, P, QDIM).transpose(1, 0, 2).reshape(P, 8 * QDIM))
    keycol = np.ascontiguousarray(key_feats[:, 0, :].reshape(NCORES, 8, P).transpose(0, 2, 1))
    vwt = np.ascontiguousarray(v_w.T).astype(ml_dtypes.bfloat16)
    vb_row = v_b.reshape(1, DIM).copy()
    irow = np.tile(np.arange(P, dtype=np.float32), (P, 1)).copy()
    ciota2 = (2 * np.arange(P, dtype=np.uint32)).reshape(P, 1).copy()

    in_maps = []
    for i in range(NCORES):
        in_maps.append(
            {
                "feats": cluster_features[i],
                "keycol": keycol[i],
                "qw": q_w,
                "qwt": qwt,
                "qb": qb_col,
                "vwt": vwt,
                "vb": vb_row,
                "irow": irow,
                "ciota2": ciota2,
            }
        )

    res = run_bass_kernel_spmd(nc, in_maps, core_ids=list(range(NCORES)))
    _CACHE["last_results"] = res

    selected = np.concatenate(
        [res.results[i]["out_sel"] for i in range(NCORES)], axis=0
    )
    fus = np.stack(
        [res.results[i]["out_fus"][0] for i in range(NCORES)], axis=0
    )
    return selected, fus


# revision 35
# speedup vs baseline: 1.1880x; 1.0011x over previous
"""AdaptiveSelection (topk_masking) Trainium2 kernel.

Per cluster c (8 clusters, one per NeuronCore, data parallel):
  Q  = feats @ q_w.T + q_b             [N, 128]
  qk = key @ q_w.T + q_b               [1, 128]
  s  = Q @ qk.T / sqrt(128)            [N]     (scores)
  A  = softmax(s)                      [N]
  idx = top_k(A, 128)                  (descending order)
  selected = feats[idx]                [128, D]
  fusion = A.T @ (feats @ v_w.T + v_b) [D]

Device restructurings:
  * s = feats @ w + const, w = q_w.T @ qk — the const and 1/sqrt(128) scale
    do not change the ordering, and softmax shift-invariance kills the const;
    the scale is applied inside the exp activation.
  * fusion = (sum_j e_j feats_j / sum_j e_j) @ v_w.T + v_b with e = exp(s/c):
    the e-weighted feature sum accumulates in PSUM (bf16 operands) during
    streaming, so the 4096x1024x1024 V matmul disappears. The final
    (a/z) @ v_w.T runs on the PE against a host-pretransposed bf16 v_w.T.
  * top-128: per-partition top-k (vector.max/max_index on a [128, 32] score
    layout, 32 elements per partition -> empirically max 5 of the global
    top-128 share a partition; 6 kept for margin), exact global ranks of the
    768 candidates via compare+accumulate, then a one-hot x index matmul
    yields the 128 row ids in descending-score order, and one indirect DMA
    gathers those rows from HBM.
  * feats stream as [128 partitions x 2 rows x 1024] groups so each DMA
    descriptor covers 8KB contiguous on both sides.
"""

import numpy as np

import concourse.mybir as mybir
from concourse import bacc, bass, tile
from concourse.bass_utils import run_bass_kernel_spmd

NCORES = 8
NPER = 4096
DIM = 1024
QDIM = 128
TOPK = 128
P = 128
R = 2  # feature rows per partition per streamed group
NG = NPER // (P * R)  # 16 streamed groups
NT = NPER // P  # 32 score columns; col = R*g + r, global row = 256g + 2p + r
NCPP = 6  # candidates kept per partition (empirical max in top-128 is 5)
W = P * NCPP  # 768 candidates
F32 = mybir.dt.float32
BF16 = mybir.dt.bfloat16
U32 = mybir.dt.uint32

_CACHE = {}


def build_bass():
    nc = bacc.Bacc(None, target_bir_lowering=False)

    feats_e = nc.declare_dram_parameter("feats", [NPER, DIM], F32, isOutput=False)
    keycol_e = nc.declare_dram_parameter("keycol", [P, 8], F32, isOutput=False)
    qw_e = nc.declare_dram_parameter("qw", [QDIM, DIM], F32, isOutput=False)
    qwt_e = nc.declare_dram_parameter("qwt", [P, 8 * QDIM], F32, isOutput=False)
    qb_e = nc.declare_dram_parameter("qb", [QDIM, 1], F32, isOutput=False)
    # v_w.T in bf16, host-prepared: [i, o] layout so fusion contracts on PE
    vwt_e = nc.declare_dram_parameter("vwt", [DIM, DIM], BF16, isOutput=False)
    vb_e = nc.declare_dram_parameter("vb", [1, DIM], F32, isOutput=False)
    irow_e = nc.declare_dram_parameter("irow", [P, P], F32, isOutput=False)
    ciota2_e = nc.declare_dram_parameter("ciota2", [P, 1], U32, isOutput=False)
    outsel_e = nc.declare_dram_parameter("out_sel", [TOPK, DIM], F32, isOutput=True)
    outfus_e = nc.declare_dram_parameter("out_fus", [1, DIM], F32, isOutput=True)

    mm = mybir.AluOpType.mult
    add = mybir.AluOpType.add
    is_gt = mybir.AluOpType.is_gt
    is_eq = mybir.AluOpType.is_equal
    COPY = mybir.ActivationFunctionType.Copy
    EXP = mybir.ActivationFunctionType.Exp

    with tile.TileContext(nc) as tc:
        with (
            tc.tile_pool(name="const", bufs=1) as cp,
            tc.tile_pool(name="stream", bufs=4) as sp,
            tc.tile_pool(name="prodp", bufs=4) as prp,
            tc.tile_pool(name="scratch", bufs=3) as wp,
            tc.tile_pool(name="psum", bufs=2, space="PSUM") as pp,
            tc.tile_pool(name="psacc", bufs=1, space="PSUM") as pa,
        ):
            # ---------- setup: constants and small inputs ----------
            # critical-chain DMAs on the sync queue, the rest on gpsimd
            ones1 = cp.tile([1, P], F32)
            nc.vector.memset(ones1[:], 1.0)
            keycol_t = cp.tile([P, 8], F32)
            nc.sync.dma_start(out=keycol_t[:], in_=keycol_e[:, :])
            qwt_t = cp.tile([P, 8, QDIM], F32)
            for hh in range(4):
                nc.sync.dma_start(
                    out=qwt_t[:].rearrange("p c q -> p (c q)")[:, hh * 256 : (hh + 1) * 256],
                    in_=qwt_e[:, hh * 256 : (hh + 1) * 256],
                )
            qw_t = cp.tile([QDIM, DIM], F32)
            for hh in range(2):
                nc.sync.dma_start(
                    out=qw_t[:, hh * 512 : (hh + 1) * 512],
                    in_=qw_e[:, hh * 512 : (hh + 1) * 512],
                )
            qb_t = cp.tile([QDIM, 1], F32)
            nc.sync.dma_start(out=qb_t[:], in_=qb_e[:, :])
            irow_t = cp.tile([P, P], F32)
            nc.gpsimd.dma_start(out=irow_t[:], in_=irow_e[:, :])
            ciota2_t = cp.tile([P, 1], U32)
            nc.gpsimd.dma_start(out=ciota2_t[:], in_=ciota2_e[:, :])
            vb_t = cp.tile([1, DIM], F32)
            nc.gpsimd.dma_start(out=vb_t[:], in_=vb_e[:, :])
            vwt_t = cp.tile([P, 8, DIM], BF16)
            for ib in range(8):
                nc.gpsimd.dma_start(
                    out=vwt_t[:, ib, :], in_=vwt_e[ib * P : (ib + 1) * P, :]
                )

            # qk = q_w @ key + q_b entirely on PE: contraction over dim via
            # the host-transposed q_w.T chunks and the column-chunked key
            pqk = pa.tile([P, 1], F32, tag="pz")
            for cch in range(8):
                nc.tensor.matmul(
                    out=pqk[:],
                    lhsT=qwt_t[:, cch, :],
                    rhs=keycol_t[:, cch : cch + 1],
                    start=(cch == 0),
                    stop=(cch == 7),
                )
            qk_t = cp.tile([QDIM, 1], F32)
            nc.vector.tensor_add(out=qk_t[:], in0=pqk[:], in1=qb_t[:])

            # w broadcast, duplicated for the 2-row groups: wb2[p, r, d] = w[d]
            wb2 = cp.tile([P, R, DIM], F32)
            for n in range(2):
                wps = pp.tile([P, 512], F32, tag="bc")
                nc.tensor.matmul(
                    out=wps[:],
                    lhsT=qk_t[:, 0:1].to_broadcast([QDIM, P]),
                    rhs=qw_t[:, n * 512 : (n + 1) * 512],
                    start=True,
                    stop=True,
                )
                nc.vector.tensor_copy(out=wb2[:, 0, n * 512 : (n + 1) * 512], in_=wps[:])
                nc.scalar.activation(
                    out=wb2[:, 1, n * 512 : (n + 1) * 512], in_=wps[:], func=COPY
                )

            # dummy indirect gather: absorbs the one-time GpSimd dynamic-DMA
            # setup (ucode load + queue drain, ~6us) during the streamed
            # phase instead of on the tail critical path
            zidx = cp.tile([16, 1], mybir.dt.int32)
            nc.vector.memset(zidx[:], 0)
            dummy = cp.tile([16, DIM], F32)
            nc.gpsimd.indirect_dma_start(
                out=dummy[:],
                out_offset=None,
                in_=feats_e[:, :],
                in_offset=bass.IndirectOffsetOnAxis(ap=zidx[:, 0:1], axis=0),
            )

            # ---------- streaming phase over 16 groups of 256 rows ----------
            # group g: partition p holds DRAM rows 256g + 2p + {0,1} (8KB
            # contiguous per partition on both sides of the DMA). Score
            # column for (g, r) is 2g + r.
            scores = cp.tile([P, NT], F32)
            evb = cp.tile([P, NT], BF16)
            pa0 = pa.tile([1, 512], F32, tag="pa0")
            pa1 = pa.tile([1, 512], F32, tag="pa1")
            pz = pa.tile([1, 1], F32, tag="pz")

            for g in range(NG):
                ft = sp.tile([P, R, DIM], F32, tag="feats")
                nc.sync.dma_start(
                    out=ft[:],
                    in_=feats_e[g * P * R : (g + 1) * P * R, :].rearrange(
                        "(p r) d -> p r d", r=R
                    ),
                )
                # bf16 cast split 3:1 between DVE and ACT to balance engines
                ftb = sp.tile([P, R, DIM], BF16, tag="featsb")
                nc.vector.tensor_copy(
                    out=ftb[:].rearrange("p r d -> p (r d)")[:, 0:1536],
                    in_=ft[:].rearrange("p r d -> p (r d)")[:, 0:1536],
                )
                nc.scalar.activation(
                    out=ftb[:].rearrange("p r d -> p (r d)")[:, 1536:2048],
                    in_=ft[:].rearrange("p r d -> p (r d)")[:, 1536:2048],
                    func=COPY,
                )
                prod = prp.tile([P, R, DIM], F32, tag="prod")
                nc.vector.tensor_tensor(out=prod[:], in0=ft[:], in1=wb2[:], op=mm)
                pact = wp.tile([P, R, DIM], F32, tag="actout")
                for r in range(R):
                    nc.scalar.activation(
                        out=pact[:, r, :],
                        in_=prod[:, r, :],
                        func=COPY,
                        accum_out=scores[:, R * g + r : R * g + r + 1],
                    )
                # softmax weight: exp(s / sqrt(QDIM)), batched over the group
                nc.scalar.activation(
                    out=evb[:, R * g : R * g + R],
                    in_=scores[:, R * g : R * g + R],
                    func=EXP,
                    scale=float(1.0 / np.sqrt(QDIM)),
                )
                # PSUM accumulation of e-weighted features (bf16 in, f32 acc)
                for r in range(R):
                    col = R * g + r
                    nc.tensor.matmul(
                        out=pa0[:],
                        lhsT=evb[:, col : col + 1],
                        rhs=ftb[:, r, 0:512],
                        start=(col == 0),
                        stop=(col == NT - 1),
                    )
                    nc.tensor.matmul(
                        out=pa1[:],
                        lhsT=evb[:, col : col + 1],
                        rhs=ftb[:, r, 512:1024],
                        start=(col == 0),
                        stop=(col == NT - 1),
                    )

            # ---------- fusion vector (placed early so PE/DMA overlap the
            # top-k phase): fusion = (a/z) @ v_w.T + v_b ----------
            zc = cp.tile([P, 1], F32)
            zact = wp.tile([P, NT], F32, tag="gts")
            nc.scalar.activation(out=zact[:], in_=evb[:], func=COPY, accum_out=zc[:])
            onescol = cp.tile([P, 1], F32)
            nc.vector.memset(onescol[:], 1.0)
            nc.tensor.matmul(
                out=pz[:], lhsT=zc[:], rhs=onescol[:], start=True, stop=True
            )
            rz = cp.tile([1, 1], F32)
            nc.vector.reciprocal(out=rz[:], in_=pz[:])
            a_sb = cp.tile([1, DIM], F32)
            nc.vector.tensor_scalar_mul(a_sb[:, 0:512], pa0[:], rz[:, 0:1])
            nc.vector.tensor_scalar_mul(a_sb[:, 512:1024], pa1[:], rz[:, 0:1])
            # a as a column-chunked [128, 8] layout (i = c*128 + p)
            acol = cp.tile([P, 8], F32)
            for c in range(8):
                nc.sync.dma_start(
                    out=acol[:, c : c + 1], in_=a_sb[:, c * P : (c + 1) * P]
                )
            acolb = cp.tile([P, 8], BF16)
            nc.vector.tensor_copy(out=acolb[:], in_=acol[:])
            pfus0 = pa.tile([1, 512], F32, tag="pfus0")
            pfus1 = pa.tile([1, 512], F32, tag="pfus1")
            for ib in range(8):
                nc.tensor.matmul(
                    out=pfus0[:],
                    lhsT=acolb[:, ib : ib + 1],
                    rhs=vwt_t[:, ib, 0:512],
                    start=(ib == 0),
                    stop=(ib == 7),
                )
                nc.tensor.matmul(
                    out=pfus1[:],
                    lhsT=acolb[:, ib : ib + 1],
                    rhs=vwt_t[:, ib, 512:1024],
                    start=(ib == 0),
                    stop=(ib == 7),
                )
            # ---------- top-k: per-partition top-6 candidates ----------
            top8 = cp.tile([P, 8], F32)
            nc.vector.max(out=top8[:], in_=scores[:])
            idx8 = cp.tile([P, 8], U32)
            nc.vector.max_index(out=idx8[:], in_max=top8[:], in_values=scores[:])
            # global row id: col -> 256*(col>>1) + (col&1) + 2p
            gidx_u = cp.tile([P, 8], U32)
            sh = cp.tile([P, 8], U32)
            nc.vector.tensor_scalar(
                sh[:], idx8[:], 1, scalar2=None,
                op0=mybir.AluOpType.logical_shift_right,
            )
            nc.vector.tensor_scalar(
                sh[:], sh[:], 8, scalar2=None,
                op0=mybir.AluOpType.logical_shift_left,
            )
            nc.vector.tensor_scalar(
                gidx_u[:], idx8[:], 1, scalar2=None,
                op0=mybir.AluOpType.bitwise_and,
            )
            nc.vector.tensor_add(out=gidx_u[:], in0=gidx_u[:], in1=sh[:])
            nc.vector.tensor_add(
                out=gidx_u[:], in0=gidx_u[:], in1=ciota2_t[:, 0:1].to_broadcast([P, 8])
            )
            gidxf = cp.tile([P, 8], F32)
            nc.vector.tensor_copy(out=gidxf[:], in_=gidx_u[:])

            # ---------- exact global ranks of the W candidates ----------
            cf = cp.tile([1, W], F32)
            nc.sync.dma_start(out=cf[:], in_=top8[:, 0:NCPP])
            rs = cp.tile([P, W], F32)
            for n in range(2):
                lo = n * 512
                hi = min(W, lo + 512)
                if lo >= hi:
                    break
                rps = pp.tile([P, 512], F32, tag="bc")
                nc.tensor.matmul(
                    out=rps[:, 0 : hi - lo],
                    lhsT=ones1[:],
                    rhs=cf[:, lo:hi],
                    start=True,
                    stop=True,
                )
                nc.vector.tensor_copy(out=rs[:, lo:hi], in_=rps[:, 0 : hi - lo])
            # rank -> one-hot -> ordered-id matmul, pipelined per candidate col
            rank = cp.tile([P, NCPP], F32)
            po = pa.tile([P, 1], F32, tag="po")
            for c in range(NCPP):
                gts = wp.tile([P, W], F32, tag="gts")
                nc.vector.tensor_tensor(
                    out=gts[:],
                    in0=rs[:],
                    in1=top8[:, c : c + 1].to_broadcast([P, W]),
                    op=is_gt,
                )
                gact = wp.tile([P, W], F32, tag="gact")
                nc.scalar.activation(
                    out=gact[:], in_=gts[:], func=COPY,
                    accum_out=rank[:, c : c + 1],
                )
                oh = wp.tile([P, P], F32, tag="oh")
                nc.vector.tensor_tensor(
                    out=oh[:],
                    in0=rank[:, c : c + 1].to_broadcast([P, P]),
                    in1=irow_t[:],
                    op=is_eq,
                )
                nc.tensor.matmul(
                    out=po[:],
                    lhsT=oh[:],
                    rhs=gidxf[:, c : c + 1],
                    start=(c == 0),
                    stop=(c == NCPP - 1),
                )
            oidx = cp.tile([P, 1], mybir.dt.int32)
            nc.vector.tensor_copy(out=oidx[:], in_=po[:])

            # ---------- gather selected rows from HBM (two half-gathers on
            # base-partition-0 tiles so transfers and writebacks overlap) ----
            oidx_b = cp.tile([64, 1], mybir.dt.int32)
            nc.sync.dma_start(out=oidx_b[:], in_=oidx[64:128, 0:1])
            sel_a = cp.tile([64, DIM], F32)
            sel_b = cp.tile([64, DIM], F32)
            nc.gpsimd.indirect_dma_start(
                out=sel_a[:],
                out_offset=None,
                in_=feats_e[:, :],
                in_offset=bass.IndirectOffsetOnAxis(ap=oidx[0:64, 0:1], axis=0),
            )
            nc.gpsimd.indirect_dma_start(
                out=sel_b[:],
                out_offset=None,
                in_=feats_e[:, :],
                in_offset=bass.IndirectOffsetOnAxis(ap=oidx_b[:, 0:1], axis=0),
            )
            for qq in range(2):
                nc.sync.dma_start(
                    out=outsel_e[qq * 32 : (qq + 1) * 32, :],
                    in_=sel_a[qq * 32 : (qq + 1) * 32, :],
                )
            for qq in range(2):
                nc.sync.dma_start(
                    out=outsel_e[64 + qq * 32 : 64 + (qq + 1) * 32, :],
                    in_=sel_b[qq * 32 : (qq + 1) * 32, :],
                )

            # fusion finalize (off the critical path, after the gather issue)
            fus = cp.tile([1, DIM], F32)
            nc.vector.tensor_copy(out=fus[:, 0:512], in_=pfus0[:])
            nc.vector.tensor_copy(out=fus[:, 512:1024], in_=pfus1[:])
            nc.vector.tensor_add(out=fus[:], in0=fus[:], in1=vb_t[:])
            nc.sync.dma_start(out=outfus_e[:, :], in_=fus[:])

    nc.finalize()
    return nc


def kernel(cluster_features, key_feats, q_w, q_b, v_w, v_b):
    import ml_dtypes

    cluster_features = np.ascontiguousarray(cluster_features, dtype=np.float32)
    key_feats = np.ascontiguousarray(key_feats, dtype=np.float32)
    q_w = np.ascontiguousarray(q_w, dtype=np.float32)
    q_b = np.ascontiguousarray(q_b, dtype=np.float32)
    v_w = np.ascontiguousarray(v_w, dtype=np.float32)
    v_b = np.ascontiguousarray(v_b, dtype=np.float32)

    if "nc" not in _CACHE:
        _CACHE["nc"] = build_bass()
    nc = _CACHE["nc"]

    qb_col = q_b.reshape(QDIM, 1).copy()
    qwt = np.ascontiguousarray(q_w.T.reshape(8, P, QDIM).transpose(1, 0, 2).reshape(P, 8 * QDIM))
    keycol = np.ascontiguousarray(key_feats[:, 0, :].reshape(NCORES, 8, P).transpose(0, 2, 1))
    vwt = np.ascontiguousarray(v_w.T).astype(ml_dtypes.bfloat16)
    vb_row = v_b.reshape(1, DIM).copy()
    irow = np.tile(np.arange(P, dtype=np.float32), (P, 1)).copy()
    ciota2 = (2 * np.arange(P, dtype=np.uint32)).reshape(P, 1).copy()

    in_maps = []
    for i in range(NCORES):
        in_maps.append(
            {
                "feats": cluster_features[i],
                "keycol": keycol[i],
                "qw": q_w,
                "qwt": qwt,
                "qb": qb_col,
                "vwt": vwt,
                "vb": vb_row,
                "irow": irow,
                "ciota2": ciota2,
            }
        )

    res = run_bass_kernel_spmd(nc, in_maps, core_ids=list(range(NCORES)))
    _CACHE["last_results"] = res

    selected = np.concatenate(
        [res.results[i]["out_sel"] for i in range(NCORES)], axis=0
    )
    fus = np.stack(
        [res.results[i]["out_fus"][0] for i in range(NCORES)], axis=0
    )
    return selected, fus
